# revision 28
# baseline (speedup 1.0000x reference)
"""Trainium2 Bass kernel for single-head attention model.

Reference computation (B=4, S=2048, E=1024, fp32):
    q = query @ Wq + bq;  k = key @ Wk + bk;  v = value @ Wv + bv
    scores = (q @ k^T) / sqrt(E)
    out = softmax(scores, axis=-1) @ v

Sharding: 8 cores; core c handles batch b = c // 2, query-row half
h = c % 2 (1024 q-rows). No collectives.

Algebraic restructure (saves ~23% of the MACs vs the direct form):
    scores_ij = x^q_i A x^k_j + g.x^k_j (+ row-const terms that cancel
    in softmax), where A = Wq Wk^T and g = Wk bq (host-computed).
    bk drops out entirely.  On the value side,
    out = softmax(scores) @ (Xv Wv + bv) = (attn @ Xv) @ Wv + bv
    since attn rows sum to 1 — Wv is applied to only the core's own
    1024 q rows instead of all 2048 kv rows.

Per-core matmul work (128x128 PE, 1 cycle/row at free>=256):
    A = WqWk^T (65536 cyc) ; q'T = A^T-contract with xqT (65536)
    scoresT (131072) ; Z^T = Xv^T-contract with exp (131072)
    O = Z Wv (65536)  => 458752 cycles ~= 191us @2.4GHz.

All matmul inputs are bf16 (host-converted); PSUM accumulates f32.
exp/Z intermediates stored bf16.  Softmax sums: DVE partial-sum chain
over the 16 key tiles, then a 256-wide ones-matmul per 128-row slice
for the partition reduction (wide enough not to break the PE p-state
stretch).  A 14-matmul PE warm-up bridges the initial DMA lead-in so
real matmuls start at full clock.  One shared 8-tag PSUM pool spans
all phases (no pool release/alloc barriers).
"""

import sys

sys.path.insert(0, "/opt/trn_rl_repo")

from contextlib import ExitStack

import ml_dtypes
import numpy as np

import concourse.mybir as mybir
import concourse.tile as tile
from concourse import bacc, bass_utils

BF16 = mybir.dt.bfloat16
F32 = mybir.dt.float32
AF = mybir.ActivationFunctionType

B, S, E = 4, 2048, 1024
N_CORES = 8
SQ = S // 2          # q rows per core
BQ = 512             # s_q block width in attention phase
NBLK = SQ // BQ      # 2 blocks
EK = E // 128        # 8 tiles over e/a/c dims
MK = S // 128        # 16 s_k tiles
INV_SCALE = 1.0 / float(np.sqrt(E))

_cached = {}


def _build():
    nc = bacc.Bacc("TRN2", target_bir_lowering=False, debug=False,
                   num_devices=N_CORES)

    # host pre-transposed / pre-converted inputs (all bf16 except consts)
    wqT = nc.dram_tensor("wqT", [E, E], BF16, kind="ExternalInput").ap()
    wkT = nc.dram_tensor("wkT", [E, E], BF16, kind="ExternalInput").ap()
    xqT = nc.dram_tensor("xqT", [E, SQ], BF16, kind="ExternalInput").ap()
    xkT = nc.dram_tensor("xkT", [E, S], BF16, kind="ExternalInput").ap()
    xv = nc.dram_tensor("xv", [S, E], BF16, kind="ExternalInput").ap()
    wv = nc.dram_tensor("wv", [E, E], BF16, kind="ExternalInput").ap()
    # g = Wk @ bq arranged g_h[p, t] = g[t*128 + p]
    gh = nc.dram_tensor("gh", [128, EK], F32, kind="ExternalInput").ap()
    bvh = nc.dram_tensor("bvh", [1, E], F32, kind="ExternalInput").ap()
    out = nc.dram_tensor("out", [SQ, E], F32, kind="ExternalOutput").ap()

    with tile.TileContext(nc) as tc, ExitStack() as top:
        # ---- long-lived pools ----
        consts = top.enter_context(tc.tile_pool(name="consts", bufs=1))
        qtpool = top.enter_context(tc.tile_pool(name="qtpool", bufs=1))
        xkpool = top.enter_context(tc.tile_pool(name="xkpool", bufs=1))
        xvpool = top.enter_context(tc.tile_pool(name="xvpool", bufs=1))
        wvpool = top.enter_context(tc.tile_pool(name="wvpool", bufs=1))

        # single shared PSUM pool: 8 tags x [128,512]f32 = 8 banks; shared
        # tags across phases avoid pool release/alloc barriers entirely
        psp = top.enter_context(tc.tile_pool(name="psp", bufs=1, space="PSUM"))

        qt_tiles = [qtpool.tile([128, SQ], BF16, tag=f"qt{m}", name=f"qt{m}")
                    for m in range(EK)]
        xk_tiles = [xkpool.tile([128, S], BF16, tag=f"xk{k}", name=f"xk{k}")
                    for k in range(EK)]
        xv_tiles = [xvpool.tile([128, E], BF16, tag=f"xv{m}", name=f"xv{m}")
                    for m in range(MK)]
        wv_tiles = [wvpool.tile([128, E], BF16, tag=f"wv{k}", name=f"wv{k}")
                    for k in range(EK)]

        with tc.tile_pool(name="wqwk", bufs=1) as wqwkp, \
             tc.tile_pool(name="apool", bufs=1) as apool, \
             tc.tile_pool(name="xqpool", bufs=1) as xqpool:
            wq_t = [wqwkp.tile([128, E], BF16, tag=f"wq{c}", name=f"wq{c}")
                    for c in range(EK)]
            wk_t = [wqwkp.tile([128, E], BF16, tag=f"wk{c}", name=f"wk{c}")
                    for c in range(EK)]
            a_tiles = [apool.tile([128, E], BF16, tag=f"a{t}", name=f"a{t}")
                       for t in range(EK)]
            xq_tiles = [xqpool.tile([128, SQ], BF16, tag=f"xq{t}", name=f"xq{t}")
                        for t in range(EK)]

            ones_r = consts.tile([128, 256], F32)
            ones_f32r = ones_r[:].bitcast(mybir.dt.float32r)

            # ---- PE warm-up: keep the tensor engine busy through the DMA
            # lead-in so the p-state ramp completes before real work.
            # Reads ones_r UNINITIALIZED on purpose (values never consumed);
            # the memset below is WAR-ordered after the warm-up reads and
            # completes long before the sums-matmuls need real ones. ----
            warm = psp.tile([128, 256], F32, tag="ps0", name="warm")
            for _ in range(15):
                nc.tensor.matmul(warm[:], ones_f32r[:, 0:128],
                                 ones_f32r, start=True, stop=True)
            nc.vector.memset(ones_r[:], 1.0)

            # ---- DMA issue order = consumption order ----
            # wq full tiles + wk first halves feed phase A's nb=0 wave
            for c in range(EK):
                nc.sync.dma_start(wq_t[c][:], wqT[c * 128:(c + 1) * 128, :])
                nc.sync.dma_start(wk_t[c][:, 0:512],
                                  wkT[c * 128:(c + 1) * 128, 0:512])
            for c in range(EK):
                nc.sync.dma_start(wk_t[c][:, 512:1024],
                                  wkT[c * 128:(c + 1) * 128, 512:1024])
            g_t = consts.tile([128, EK], F32)
            nc.sync.dma_start(g_t[:], gh)
            bv_row = consts.tile([1, E], F32)
            nc.sync.dma_start(bv_row[:], bvh)
            bv_bc = consts.tile([128, E], F32)
            nc.gpsimd.partition_broadcast(bv_bc[:], bv_row[:])
            for t in range(EK):
                nc.sync.dma_start(xq_tiles[t][:], xqT[t * 128:(t + 1) * 128, :])
            for k in range(EK):
                nc.sync.dma_start(xk_tiles[k][:], xkT[k * 128:(k + 1) * 128, :])
            for m in range(MK):
                nc.sync.dma_start(xv_tiles[m][:], xv[m * 128:(m + 1) * 128, :])
            for k in range(EK):
                nc.sync.dma_start(wv_tiles[k][:], wv[k * 128:(k + 1) * 128, :])

            # ====== phase A: A = Wq Wk^T  (c-outer PSUM waves; the final
            # half-waves let next-phase matmuls overlap the copy tail) ======
            def a_wave(nb, ts_):
                psa = {t: psp.tile([128, 512], F32, tag=f"ps{t}",
                                   name=f"psA{nb}_{t}") for t in ts_}
                for c in range(EK):
                    for t in ts_:
                        nc.tensor.matmul(
                            psa[t][:],
                            wq_t[c][:, t * 128:(t + 1) * 128],
                            wk_t[c][:, nb * 512:(nb + 1) * 512],
                            start=(c == 0), stop=(c == EK - 1))
                # drain copies split across DVE/Act
                for t in ts_:
                    dst = a_tiles[t][:, nb * 512:(nb + 1) * 512]
                    if t % 2 == 0:
                        nc.vector.tensor_scalar_add(dst, psa[t][:], 0.0)
                    else:
                        nc.scalar.copy(dst, psa[t][:])

            a_wave(0, range(8))
            a_wave(1, range(0, 4))
            a_wave(1, range(4, 8))

            # ====== phase Q: q''T = A^T-contraction with xqT, + g ======
            def q_wave(nb, ms_):
                psq = {m: psp.tile([128, 512], F32, tag=f"ps{m}",
                                   name=f"psQ{nb}_{m}") for m in ms_}
                for t in range(EK):
                    for m in ms_:
                        nc.tensor.matmul(
                            psq[m][:],
                            a_tiles[t][:, m * 128:(m + 1) * 128],
                            xq_tiles[t][:, nb * 512:(nb + 1) * 512],
                            start=(t == 0), stop=(t == EK - 1))
                for m in ms_:
                    dst = qt_tiles[m][:, nb * 512:(nb + 1) * 512]
                    if m % 2 == 0:
                        nc.vector.tensor_scalar_add(
                            dst, psq[m][:], g_t[:, m:m + 1])
                    else:
                        nc.scalar.activation(
                            dst, psq[m][:], AF.Identity,
                            bias=g_t[:, m:m + 1])

            q_wave(0, range(8))
            q_wave(1, range(0, 4))
            q_wave(1, range(4, 8))

        # ====== phase D: attention, blocked over s_q ======
        with tc.tile_pool(name="expp", bufs=1) as expp, \
             tc.tile_pool(name="ztp", bufs=2) as ztp, \
             tc.tile_pool(name="otp", bufs=1) as otp, \
             tc.tile_pool(name="partp", bufs=2) as partp, \
             tc.tile_pool(name="rcp", bufs=2) as rcp:
            for blk in range(NBLK):
                q0 = blk * BQ
                # scoresT[s_k, blk] -> exp (bf16)
                exps = []
                for m in range(MK):
                    ps = psp.tile([128, BQ], F32, tag=f"ps{m % 2}",
                                  name=f"psS{blk}_{m}")
                    for k in range(EK):
                        nc.tensor.matmul(
                            ps[:],
                            xk_tiles[k][:, m * 128:(m + 1) * 128],
                            qt_tiles[k][:, q0:q0 + BQ],
                            start=(k == 0), stop=(k == EK - 1))
                    et = expp.tile([128, BQ], BF16, tag=f"exp{m}",
                                   name=f"exp{blk}_{m}")
                    nc.scalar.activation(et[:], ps[:], AF.Exp, scale=INV_SCALE)
                    exps.append(et)

                # partial sums over s_k tiles (DVE chain), overlaps ZT below
                part = partp.tile([128, BQ], F32, tag="part",
                                  name=f"part{blk}")
                nc.vector.tensor_add(part[:], exps[0][:], exps[1][:])
                for m in range(2, MK - 1):
                    nc.vector.tensor_add(part[:], part[:], exps[m][:])
                part_r = partp.tile([128, BQ], mybir.dt.float32r, tag="part_r",
                                    name=f"part_r{blk}")
                nc.vector.tensor_add(part_r[:], part[:], exps[MK - 1][:])

                # Z^T[e, i] = sum_j Xv[j, e] expT[j, i]
                zts = []
                for e_ in range(EK):
                    ps = psp.tile([128, BQ], F32, tag=f"ps{2 + e_ % 2}",
                                  name=f"psZ{blk}_{e_}")
                    for m in range(MK):
                        nc.tensor.matmul(
                            ps[:],
                            xv_tiles[m][:, e_ * 128:(e_ + 1) * 128],
                            exps[m][:],
                            start=(m == 0), stop=(m == MK - 1))
                    zt = ztp.tile([128, BQ], BF16, tag=f"zt{e_}",
                                  name=f"zt{blk}_{e_}")
                    nc.scalar.copy(zt[:], ps[:])
                    zts.append(zt)

                # partition-reduce of part_r via 256-wide ones-matmuls
                # (real-size matmuls keep the PE p-state stretch alive)
                recips = []
                for sh in range(2):
                    pssum = psp.tile([128, 512], F32, tag=f"ps{6 + sh}",
                                     name=f"psSum{blk}_{sh}")
                    for sl in range(2):
                        s = sh * 2 + sl
                        nc.tensor.matmul(
                            pssum[:, sl * 256:(sl + 1) * 256],
                            part_r[:, s * 128:(s + 1) * 128],
                            ones_r[:].bitcast(mybir.dt.float32r),
                            start=True, stop=True)
                    for sl in range(2):
                        s = sh * 2 + sl
                        rc = rcp.tile([128, 1], F32, tag=f"rc{s}",
                                      name=f"rc{blk}_{s}")
                        nc.vector.reciprocal(
                            rc[:], pssum[:, sl * 256:sl * 256 + 1])
                        recips.append(rc)

                # O = Z @ Wv, normalize by recip, + bv, DMA out.
                # The very last i-tile is split into 256-wide chunks spread
                # through the section so its trailing DMA chain is shallow.
                if blk == NBLK - 1:
                    order = [(0, 0, 512), (0, 512, 512),
                             (1, 0, 512), (1, 512, 512),
                             (3, 0, 256), (3, 256, 256),
                             (2, 0, 512), (2, 512, 512),
                             (3, 512, 256), (3, 768, 256)]
                else:
                    order = [(it, n * 512, 512) for it in range(BQ // 128)
                             for n in range(2)]
                ots = {}
                for ci, (it, f0, cw) in enumerate(order):
                    if it not in ots:
                        ots[it] = otp.tile([128, E], F32, tag=f"ot{it}",
                                           name=f"ot{blk}_{it}")
                    ot = ots[it]
                    ps = psp.tile([128, cw], F32, tag=f"ps{4 + ci % 2}",
                                  name=f"psO{blk}_{it}_{ci}")
                    for e_ in range(EK):
                        nc.tensor.matmul(
                            ps[:],
                            zts[e_][:, it * 128:(it + 1) * 128],
                            wv_tiles[e_][:, f0:f0 + cw],
                            start=(e_ == 0), stop=(e_ == EK - 1))
                    nc.scalar.activation(
                        ot[:, f0:f0 + cw], ps[:],
                        AF.Copy, scale=recips[it][:])
                    nc.vector.tensor_add(
                        ot[:, f0:f0 + cw],
                        ot[:, f0:f0 + cw],
                        bv_bc[:, f0:f0 + cw])
                    nc.sync.dma_start(
                        out[q0 + it * 128:q0 + (it + 1) * 128,
                            f0:f0 + cw],
                        ot[:, f0:f0 + cw])

    nc.compile()
    return nc


def _get_nc():
    if "nc" not in _cached:
        _cached["nc"] = _build()
    return _cached["nc"]


def _bf16(a):
    return np.ascontiguousarray(np.asarray(a, dtype=np.float32)).astype(
        ml_dtypes.bfloat16)


def kernel(query, key, value, Wq, bq, Wk, bk, Wv, bv, **kw):
    query = np.asarray(query, dtype=np.float32)
    key = np.asarray(key, dtype=np.float32)
    value = np.asarray(value, dtype=np.float32)
    Wq = np.asarray(Wq, dtype=np.float32)
    Wk = np.asarray(Wk, dtype=np.float32)
    Wv = np.asarray(Wv, dtype=np.float32)
    bq = np.asarray(bq, dtype=np.float32)
    bv = np.asarray(bv, dtype=np.float32)

    wqT_h = _bf16(Wq.T)
    wkT_h = _bf16(Wk.T)
    wv_h = _bf16(Wv)
    g = Wk @ bq                       # [E]; bk cancels in softmax
    g_h = np.ascontiguousarray(g.reshape(EK, 128).T).astype(np.float32)
    bv_h = np.ascontiguousarray(bv.reshape(1, E))

    keyT = {b: _bf16(key[b].T) for b in range(B)}
    valN = {b: _bf16(value[b]) for b in range(B)}

    in_maps = []
    for c in range(N_CORES):
        b, h = divmod(c, 2)
        qT = _bf16(query[b, h * SQ:(h + 1) * SQ, :].T)
        in_maps.append({
            "wqT": wqT_h, "wkT": wkT_h, "xqT": qT,
            "xkT": keyT[b], "xv": valN[b], "wv": wv_h,
            "gh": g_h, "bvh": bv_h,
        })

    nc = _get_nc()
    res = bass_utils.run_bass_kernel_spmd(
        nc, in_maps, core_ids=list(range(N_CORES)), **kw)

    full = np.empty((B, S, E), dtype=np.float32)
    for c in range(N_CORES):
        b, h = divmod(c, 2)
        full[b, h * SQ:(h + 1) * SQ, :] = res.results[c]["out"]
    kernel.last_results = res
    return full


# revision 29
# speedup vs baseline: 1.0005x; 1.0005x over previous
"""Trainium2 Bass kernel for single-head attention model.

Reference computation (B=4, S=2048, E=1024, fp32):
    q = query @ Wq + bq;  k = key @ Wk + bk;  v = value @ Wv + bv
    scores = (q @ k^T) / sqrt(E)
    out = softmax(scores, axis=-1) @ v

Sharding: 8 cores; core c handles batch b = c // 2, query-row half
h = c % 2 (1024 q-rows). No collectives.

Algebraic restructure (saves ~23% of the MACs vs the direct form):
    scores_ij = x^q_i A x^k_j + g.x^k_j (+ row-const terms that cancel
    in softmax), where A = Wq Wk^T and g = Wk bq (host-computed).
    bk drops out entirely.  On the value side,
    out = softmax(scores) @ (Xv Wv + bv) = (attn @ Xv) @ Wv + bv
    since attn rows sum to 1 — Wv is applied to only the core's own
    1024 q rows instead of all 2048 kv rows.

Per-core matmul work (128x128 PE, 1 cycle/row at free>=256):
    A = WqWk^T (65536 cyc) ; q'T = A^T-contract with xqT (65536)
    scoresT (131072) ; Z^T = Xv^T-contract with exp (131072)
    O = Z Wv (65536)  => 458752 cycles ~= 191us @2.4GHz.

All matmul inputs are bf16 (host-converted); PSUM accumulates f32.
exp/Z intermediates stored bf16.  Softmax sums: DVE partial-sum chain
over the 16 key tiles, then a 256-wide ones-matmul per 128-row slice
for the partition reduction (wide enough not to break the PE p-state
stretch).  A 14-matmul PE warm-up bridges the initial DMA lead-in so
real matmuls start at full clock.  One shared 8-tag PSUM pool spans
all phases (no pool release/alloc barriers).
"""

import sys

sys.path.insert(0, "/opt/trn_rl_repo")

from contextlib import ExitStack

import ml_dtypes
import numpy as np

import concourse.mybir as mybir
import concourse.tile as tile
from concourse import bacc, bass_utils

BF16 = mybir.dt.bfloat16
F32 = mybir.dt.float32
AF = mybir.ActivationFunctionType

B, S, E = 4, 2048, 1024
N_CORES = 8
SQ = S // 2          # q rows per core
BQ = 512             # s_q block width in attention phase
NBLK = SQ // BQ      # 2 blocks
EK = E // 128        # 8 tiles over e/a/c dims
MK = S // 128        # 16 s_k tiles
INV_SCALE = 1.0 / float(np.sqrt(E))

_cached = {}


def _build():
    nc = bacc.Bacc("TRN2", target_bir_lowering=False, debug=False,
                   num_devices=N_CORES)

    # host pre-transposed / pre-converted inputs (all bf16 except consts)
    wqT = nc.dram_tensor("wqT", [E, E], BF16, kind="ExternalInput").ap()
    wkT = nc.dram_tensor("wkT", [E, E], BF16, kind="ExternalInput").ap()
    xqT = nc.dram_tensor("xqT", [E, SQ], BF16, kind="ExternalInput").ap()
    xkT = nc.dram_tensor("xkT", [E, S], BF16, kind="ExternalInput").ap()
    xv = nc.dram_tensor("xv", [S, E], BF16, kind="ExternalInput").ap()
    wv = nc.dram_tensor("wv", [E, E], BF16, kind="ExternalInput").ap()
    # g = Wk @ bq arranged g_h[p, t] = g[t*128 + p]
    gh = nc.dram_tensor("gh", [128, EK], F32, kind="ExternalInput").ap()
    bvh = nc.dram_tensor("bvh", [1, E], F32, kind="ExternalInput").ap()
    out = nc.dram_tensor("out", [SQ, E], F32, kind="ExternalOutput").ap()

    with tile.TileContext(nc) as tc, ExitStack() as top:
        # ---- long-lived pools ----
        consts = top.enter_context(tc.tile_pool(name="consts", bufs=1))
        qtpool = top.enter_context(tc.tile_pool(name="qtpool", bufs=1))
        xkpool = top.enter_context(tc.tile_pool(name="xkpool", bufs=1))
        xvpool = top.enter_context(tc.tile_pool(name="xvpool", bufs=1))
        wvpool = top.enter_context(tc.tile_pool(name="wvpool", bufs=1))

        # single shared PSUM pool: 8 tags x [128,512]f32 = 8 banks; shared
        # tags across phases avoid pool release/alloc barriers entirely
        psp = top.enter_context(tc.tile_pool(name="psp", bufs=1, space="PSUM"))

        qt_tiles = [qtpool.tile([128, SQ], BF16, tag=f"qt{m}", name=f"qt{m}")
                    for m in range(EK)]
        xk_tiles = [xkpool.tile([128, S], BF16, tag=f"xk{k}", name=f"xk{k}")
                    for k in range(EK)]
        xv_tiles = [xvpool.tile([128, E], BF16, tag=f"xv{m}", name=f"xv{m}")
                    for m in range(MK)]
        wv_tiles = [wvpool.tile([128, E], BF16, tag=f"wv{k}", name=f"wv{k}")
                    for k in range(EK)]

        with tc.tile_pool(name="wqwk", bufs=1) as wqwkp, \
             tc.tile_pool(name="apool", bufs=1) as apool, \
             tc.tile_pool(name="xqpool", bufs=1) as xqpool:
            wq_t = [wqwkp.tile([128, E], BF16, tag=f"wq{c}", name=f"wq{c}")
                    for c in range(EK)]
            wk_t = [wqwkp.tile([128, E], BF16, tag=f"wk{c}", name=f"wk{c}")
                    for c in range(EK)]
            a_tiles = [apool.tile([128, E], BF16, tag=f"a{t}", name=f"a{t}")
                       for t in range(EK)]
            xq_tiles = [xqpool.tile([128, SQ], BF16, tag=f"xq{t}", name=f"xq{t}")
                        for t in range(EK)]

            ones_r = consts.tile([128, 256], F32)
            ones_f32r = ones_r[:].bitcast(mybir.dt.float32r)

            # ---- PE warm-up: keep the tensor engine busy through the DMA
            # lead-in so the p-state ramp completes before real work.
            # Reads ones_r UNINITIALIZED on purpose (values never consumed);
            # the memset below is WAR-ordered after the warm-up reads and
            # completes long before the sums-matmuls need real ones. ----
            warm = psp.tile([128, 256], F32, tag="ps0", name="warm")
            for _ in range(15):
                nc.tensor.matmul(warm[:], ones_f32r[:, 0:128],
                                 ones_f32r, start=True, stop=True)
            nc.vector.memset(ones_r[:], 1.0)

            # ---- DMA issue order = consumption order ----
            # wq full tiles + wk first halves feed phase A's nb=0 wave
            for c in range(EK):
                nc.sync.dma_start(wq_t[c][:], wqT[c * 128:(c + 1) * 128, :])
                nc.sync.dma_start(wk_t[c][:, 0:512],
                                  wkT[c * 128:(c + 1) * 128, 0:512])
            for c in range(EK):
                nc.sync.dma_start(wk_t[c][:, 512:1024],
                                  wkT[c * 128:(c + 1) * 128, 512:1024])
            g_t = consts.tile([128, EK], F32)
            nc.sync.dma_start(g_t[:], gh)
            bv_row = consts.tile([1, E], F32)
            nc.sync.dma_start(bv_row[:], bvh)
            bv_bc = consts.tile([128, E], F32)
            nc.gpsimd.partition_broadcast(bv_bc[:], bv_row[:])
            for t in range(EK):
                nc.sync.dma_start(xq_tiles[t][:], xqT[t * 128:(t + 1) * 128, :])
            for k in range(EK):
                nc.sync.dma_start(xk_tiles[k][:], xkT[k * 128:(k + 1) * 128, :])
            for m in range(MK):
                nc.sync.dma_start(xv_tiles[m][:], xv[m * 128:(m + 1) * 128, :])
            for k in range(EK):
                nc.sync.dma_start(wv_tiles[k][:], wv[k * 128:(k + 1) * 128, :])

            # ====== phase A: A = Wq Wk^T  (c-outer PSUM waves; the final
            # half-waves let next-phase matmuls overlap the copy tail) ======
            def a_wave(nb, ts_):
                psa = {t: psp.tile([128, 512], F32, tag=f"ps{t}",
                                   name=f"psA{nb}_{t}") for t in ts_}
                for c in range(EK):
                    for t in ts_:
                        nc.tensor.matmul(
                            psa[t][:],
                            wq_t[c][:, t * 128:(t + 1) * 128],
                            wk_t[c][:, nb * 512:(nb + 1) * 512],
                            start=(c == 0), stop=(c == EK - 1))
                # drain copies split across DVE/Act
                for t in ts_:
                    dst = a_tiles[t][:, nb * 512:(nb + 1) * 512]
                    if t % 2 == 0:
                        nc.vector.tensor_scalar_add(dst, psa[t][:], 0.0)
                    else:
                        nc.scalar.copy(dst, psa[t][:])

            a_wave(0, range(8))
            a_wave(1, range(0, 4))
            a_wave(1, range(4, 8))

            # ====== phase Q: q''T = A^T-contraction with xqT, + g ======
            def q_wave(nb, ms_):
                psq = {m: psp.tile([128, 512], F32, tag=f"ps{m}",
                                   name=f"psQ{nb}_{m}") for m in ms_}
                for t in range(EK):
                    for m in ms_:
                        nc.tensor.matmul(
                            psq[m][:],
                            a_tiles[t][:, m * 128:(m + 1) * 128],
                            xq_tiles[t][:, nb * 512:(nb + 1) * 512],
                            start=(t == 0), stop=(t == EK - 1))
                for m in ms_:
                    dst = qt_tiles[m][:, nb * 512:(nb + 1) * 512]
                    if m % 2 == 0:
                        nc.vector.tensor_scalar_add(
                            dst, psq[m][:], g_t[:, m:m + 1])
                    else:
                        nc.scalar.activation(
                            dst, psq[m][:], AF.Identity,
                            bias=g_t[:, m:m + 1])

            q_wave(0, range(8))
            q_wave(1, range(0, 4))
            q_wave(1, range(4, 8))

        # ====== phase D: attention, blocked over s_q ======
        with tc.tile_pool(name="expp", bufs=1) as expp, \
             tc.tile_pool(name="ztp", bufs=2) as ztp, \
             tc.tile_pool(name="otp", bufs=1) as otp, \
             tc.tile_pool(name="partp", bufs=2) as partp, \
             tc.tile_pool(name="rcp", bufs=2) as rcp:
            for blk in range(NBLK):
                q0 = blk * BQ
                # scoresT[s_k, blk] -> exp (bf16)
                exps = []
                for m in range(MK):
                    ps = psp.tile([128, BQ], F32, tag=f"ps{m % 2}",
                                  name=f"psS{blk}_{m}")
                    for k in range(EK):
                        nc.tensor.matmul(
                            ps[:],
                            xk_tiles[k][:, m * 128:(m + 1) * 128],
                            qt_tiles[k][:, q0:q0 + BQ],
                            start=(k == 0), stop=(k == EK - 1))
                    et = expp.tile([128, BQ], BF16, tag=f"exp{m}",
                                   name=f"exp{blk}_{m}")
                    nc.scalar.activation(et[:], ps[:], AF.Exp, scale=INV_SCALE)
                    exps.append(et)

                # partial sums over s_k tiles (DVE chain), overlaps ZT below
                part = partp.tile([128, BQ], F32, tag="part",
                                  name=f"part{blk}")
                nc.vector.tensor_add(part[:], exps[0][:], exps[1][:])
                for m in range(2, MK - 1):
                    nc.vector.tensor_add(part[:], part[:], exps[m][:])
                part_r = partp.tile([128, BQ], mybir.dt.float32r, tag="part_r",
                                    name=f"part_r{blk}")
                nc.vector.tensor_add(part_r[:], part[:], exps[MK - 1][:])

                # Z^T[e, i] = sum_j Xv[j, e] expT[j, i]
                zts = []
                for e_ in range(EK):
                    ps = psp.tile([128, BQ], F32, tag=f"ps{2 + e_ % 2}",
                                  name=f"psZ{blk}_{e_}")
                    for m in range(MK):
                        nc.tensor.matmul(
                            ps[:],
                            xv_tiles[m][:, e_ * 128:(e_ + 1) * 128],
                            exps[m][:],
                            start=(m == 0), stop=(m == MK - 1))
                    zt = ztp.tile([128, BQ], BF16, tag=f"zt{e_}",
                                  name=f"zt{blk}_{e_}")
                    nc.scalar.copy(zt[:], ps[:])
                    zts.append(zt)

                # partition-reduce of part_r via 256-wide ones-matmuls
                # (real-size matmuls keep the PE p-state stretch alive)
                recips = []
                for sh in range(2):
                    pssum = psp.tile([128, 512], F32, tag=f"ps{6 + sh}",
                                     name=f"psSum{blk}_{sh}")
                    for sl in range(2):
                        s = sh * 2 + sl
                        nc.tensor.matmul(
                            pssum[:, sl * 256:(sl + 1) * 256],
                            part_r[:, s * 128:(s + 1) * 128],
                            ones_r[:].bitcast(mybir.dt.float32r),
                            start=True, stop=True)
                    for sl in range(2):
                        s = sh * 2 + sl
                        rc = rcp.tile([128, 1], F32, tag=f"rc{s}",
                                      name=f"rc{blk}_{s}")
                        nc.vector.reciprocal(
                            rc[:], pssum[:, sl * 256:sl * 256 + 1])
                        recips.append(rc)

                # O = Z @ Wv, normalize by recip, + bv, DMA out.
                # The very last i-tile uses 256-wide PSUM groups so its
                # post-processing + DMAs overlap the final matmuls.
                for it in range(BQ // 128):
                    ot = otp.tile([128, E], F32, tag=f"ot{it}",
                                  name=f"ot{blk}_{it}")
                    last_it = (blk == NBLK - 1 and it == BQ // 128 - 1)
                    cw = 256 if last_it else 512
                    for ci, f0 in enumerate(range(0, E, cw)):
                        ps = psp.tile([128, cw], F32, tag=f"ps{4 + ci % 2}",
                                      name=f"psO{blk}_{it}_{ci}")
                        for e_ in range(EK):
                            nc.tensor.matmul(
                                ps[:],
                                zts[e_][:, it * 128:(it + 1) * 128],
                                wv_tiles[e_][:, f0:f0 + cw],
                                start=(e_ == 0), stop=(e_ == EK - 1))
                        nc.scalar.activation(
                            ot[:, f0:f0 + cw], ps[:],
                            AF.Copy, scale=recips[it][:])
                        nc.vector.tensor_add(
                            ot[:, f0:f0 + cw],
                            ot[:, f0:f0 + cw],
                            bv_bc[:, f0:f0 + cw])
                        nc.sync.dma_start(
                            out[q0 + it * 128:q0 + (it + 1) * 128,
                                f0:f0 + cw],
                            ot[:, f0:f0 + cw])

    nc.compile()
    return nc


def _get_nc():
    if "nc" not in _cached:
        _cached["nc"] = _build()
    return _cached["nc"]


def _bf16(a):
    return np.ascontiguousarray(np.asarray(a, dtype=np.float32)).astype(
        ml_dtypes.bfloat16)


def kernel(query, key, value, Wq, bq, Wk, bk, Wv, bv, **kw):
    query = np.asarray(query, dtype=np.float32)
    key = np.asarray(key, dtype=np.float32)
    value = np.asarray(value, dtype=np.float32)
    Wq = np.asarray(Wq, dtype=np.float32)
    Wk = np.asarray(Wk, dtype=np.float32)
    Wv = np.asarray(Wv, dtype=np.float32)
    bq = np.asarray(bq, dtype=np.float32)
    bv = np.asarray(bv, dtype=np.float32)

    wqT_h = _bf16(Wq.T)
    wkT_h = _bf16(Wk.T)
    wv_h = _bf16(Wv)
    g = Wk @ bq                       # [E]; bk cancels in softmax
    g_h = np.ascontiguousarray(g.reshape(EK, 128).T).astype(np.float32)
    bv_h = np.ascontiguousarray(bv.reshape(1, E))

    keyT = {b: _bf16(key[b].T) for b in range(B)}
    valN = {b: _bf16(value[b]) for b in range(B)}

    in_maps = []
    for c in range(N_CORES):
        b, h = divmod(c, 2)
        qT = _bf16(query[b, h * SQ:(h + 1) * SQ, :].T)
        in_maps.append({
            "wqT": wqT_h, "wkT": wkT_h, "xqT": qT,
            "xkT": keyT[b], "xv": valN[b], "wv": wv_h,
            "gh": g_h, "bvh": bv_h,
        })

    nc = _get_nc()
    res = bass_utils.run_bass_kernel_spmd(
        nc, in_maps, core_ids=list(range(N_CORES)), **kw)

    full = np.empty((B, S, E), dtype=np.float32)
    for c in range(N_CORES):
        b, h = divmod(c, 2)
        full[b, h * SQ:(h + 1) * SQ, :] = res.results[c]["out"]
    kernel.last_results = res
    return full


# revision 30
# speedup vs baseline: 1.0026x; 1.0021x over previous
"""Trainium2 Bass kernel for single-head attention model.

Reference computation (B=4, S=2048, E=1024, fp32):
    q = query @ Wq + bq;  k = key @ Wk + bk;  v = value @ Wv + bv
    scores = (q @ k^T) / sqrt(E)
    out = softmax(scores, axis=-1) @ v

Sharding: 8 cores; core c handles batch b = c // 2, query-row half
h = c % 2 (1024 q-rows). No collectives.

Algebraic restructure (saves ~23% of the MACs vs the direct form):
    scores_ij = x^q_i A x^k_j + g.x^k_j (+ row-const terms that cancel
    in softmax), where A = Wq Wk^T and g = Wk bq (host-computed).
    bk drops out entirely.  On the value side,
    out = softmax(scores) @ (Xv Wv + bv) = (attn @ Xv) @ Wv + bv
    since attn rows sum to 1 — Wv is applied to only the core's own
    1024 q rows instead of all 2048 kv rows.

Per-core matmul work (128x128 PE, 1 cycle/row at free>=256):
    A = WqWk^T (65536 cyc) ; q'T = A^T-contract with xqT (65536)
    scoresT (131072) ; Z^T = Xv^T-contract with exp (131072)
    O = Z Wv (65536)  => 458752 cycles ~= 191us @2.4GHz.

All matmul inputs are bf16 (host-converted); PSUM accumulates f32.
exp/Z intermediates stored bf16.  Softmax sums: DVE partial-sum chain
over the 16 key tiles, then a 256-wide ones-matmul per 128-row slice
for the partition reduction (wide enough not to break the PE p-state
stretch).  A 14-matmul PE warm-up bridges the initial DMA lead-in so
real matmuls start at full clock.  One shared 8-tag PSUM pool spans
all phases (no pool release/alloc barriers).
"""

import sys

sys.path.insert(0, "/opt/trn_rl_repo")

from contextlib import ExitStack

import ml_dtypes
import numpy as np

import concourse.mybir as mybir
import concourse.tile as tile
from concourse import bacc, bass_utils

BF16 = mybir.dt.bfloat16
F32 = mybir.dt.float32
AF = mybir.ActivationFunctionType

B, S, E = 4, 2048, 1024
N_CORES = 8
SQ = S // 2          # q rows per core
BQ = 512             # s_q block width in attention phase
NBLK = SQ // BQ      # 2 blocks
EK = E // 128        # 8 tiles over e/a/c dims
MK = S // 128        # 16 s_k tiles
INV_SCALE = 1.0 / float(np.sqrt(E))

_cached = {}


def _build():
    nc = bacc.Bacc("TRN2", target_bir_lowering=False, debug=False,
                   num_devices=N_CORES)

    # host pre-transposed / pre-converted inputs (all bf16 except consts)
    wqT = nc.dram_tensor("wqT", [E, E], BF16, kind="ExternalInput").ap()
    wkT = nc.dram_tensor("wkT", [E, E], BF16, kind="ExternalInput").ap()
    xqT = nc.dram_tensor("xqT", [E, SQ], BF16, kind="ExternalInput").ap()
    xkT = nc.dram_tensor("xkT", [E, S], BF16, kind="ExternalInput").ap()
    xv = nc.dram_tensor("xv", [S, E], BF16, kind="ExternalInput").ap()
    wv = nc.dram_tensor("wv", [E, E], BF16, kind="ExternalInput").ap()
    # g = Wk @ bq arranged g_h[p, t] = g[t*128 + p]
    gh = nc.dram_tensor("gh", [128, EK], F32, kind="ExternalInput").ap()
    bvh = nc.dram_tensor("bvh", [1, E], F32, kind="ExternalInput").ap()
    out = nc.dram_tensor("out", [SQ, E], F32, kind="ExternalOutput").ap()

    with tile.TileContext(nc) as tc, ExitStack() as top:
        # ---- long-lived pools ----
        consts = top.enter_context(tc.tile_pool(name="consts", bufs=1))
        qtpool = top.enter_context(tc.tile_pool(name="qtpool", bufs=1))
        xkpool = top.enter_context(tc.tile_pool(name="xkpool", bufs=1))
        xvpool = top.enter_context(tc.tile_pool(name="xvpool", bufs=1))
        wvpool = top.enter_context(tc.tile_pool(name="wvpool", bufs=1))

        # single shared PSUM pool: 8 tags x [128,512]f32 = 8 banks; shared
        # tags across phases avoid pool release/alloc barriers entirely
        psp = top.enter_context(tc.tile_pool(name="psp", bufs=1, space="PSUM"))

        qt_tiles = [qtpool.tile([128, SQ], BF16, tag=f"qt{m}", name=f"qt{m}")
                    for m in range(EK)]
        xk_tiles = [xkpool.tile([128, S], BF16, tag=f"xk{k}", name=f"xk{k}")
                    for k in range(EK)]
        xv_tiles = [xvpool.tile([128, E], BF16, tag=f"xv{m}", name=f"xv{m}")
                    for m in range(MK)]
        wv_tiles = [wvpool.tile([128, E], BF16, tag=f"wv{k}", name=f"wv{k}")
                    for k in range(EK)]

        with tc.tile_pool(name="wqwk", bufs=1) as wqwkp, \
             tc.tile_pool(name="apool", bufs=1) as apool, \
             tc.tile_pool(name="xqpool", bufs=1) as xqpool:
            wq_t = [wqwkp.tile([128, E], BF16, tag=f"wq{c}", name=f"wq{c}")
                    for c in range(EK)]
            wk_t = [wqwkp.tile([128, E], BF16, tag=f"wk{c}", name=f"wk{c}")
                    for c in range(EK)]
            a_tiles = [apool.tile([128, E], BF16, tag=f"a{t}", name=f"a{t}")
                       for t in range(EK)]
            xq_tiles = [xqpool.tile([128, SQ], BF16, tag=f"xq{t}", name=f"xq{t}")
                        for t in range(EK)]

            ones_r = consts.tile([128, 256], F32)
            ones_f32r = ones_r[:].bitcast(mybir.dt.float32r)

            # ---- PE warm-up: keep the tensor engine busy through the DMA
            # lead-in so the p-state ramp completes before real work.
            # Reads ones_r UNINITIALIZED on purpose (values never consumed);
            # the memset below is WAR-ordered after the warm-up reads and
            # completes long before the sums-matmuls need real ones. ----
            warm = psp.tile([128, 256], F32, tag="ps0", name="warm")
            for _ in range(14):
                nc.tensor.matmul(warm[:], ones_f32r[:, 0:128],
                                 ones_f32r, start=True, stop=True)
            nc.vector.memset(ones_r[:], 1.0)

            # ---- DMA issue order = consumption order ----
            # wq full tiles + wk first halves feed phase A's nb=0 wave
            for c in range(EK):
                nc.sync.dma_start(wq_t[c][:], wqT[c * 128:(c + 1) * 128, :])
                nc.sync.dma_start(wk_t[c][:, 0:512],
                                  wkT[c * 128:(c + 1) * 128, 0:512])
            for c in range(EK):
                nc.sync.dma_start(wk_t[c][:, 512:1024],
                                  wkT[c * 128:(c + 1) * 128, 512:1024])
            g_t = consts.tile([128, EK], F32)
            nc.sync.dma_start(g_t[:], gh)
            bv_row = consts.tile([1, E], F32)
            nc.sync.dma_start(bv_row[:], bvh)
            bv_bc = consts.tile([128, E], F32)
            nc.gpsimd.partition_broadcast(bv_bc[:], bv_row[:])
            for t in range(EK):
                nc.sync.dma_start(xq_tiles[t][:], xqT[t * 128:(t + 1) * 128, :])
            for k in range(EK):
                nc.sync.dma_start(xk_tiles[k][:], xkT[k * 128:(k + 1) * 128, :])
            for m in range(MK):
                nc.sync.dma_start(xv_tiles[m][:], xv[m * 128:(m + 1) * 128, :])
            for k in range(EK):
                nc.sync.dma_start(wv_tiles[k][:], wv[k * 128:(k + 1) * 128, :])

            # ====== phase A: A = Wq Wk^T  (c-outer PSUM waves; the final
            # half-waves let next-phase matmuls overlap the copy tail) ======
            def a_wave(nb, ts_):
                psa = {t: psp.tile([128, 512], F32, tag=f"ps{t}",
                                   name=f"psA{nb}_{t}") for t in ts_}
                for c in range(EK):
                    for t in ts_:
                        nc.tensor.matmul(
                            psa[t][:],
                            wq_t[c][:, t * 128:(t + 1) * 128],
                            wk_t[c][:, nb * 512:(nb + 1) * 512],
                            start=(c == 0), stop=(c == EK - 1))
                # drain copies split across DVE/Act
                for t in ts_:
                    dst = a_tiles[t][:, nb * 512:(nb + 1) * 512]
                    if t % 2 == 0:
                        nc.vector.tensor_scalar_add(dst, psa[t][:], 0.0)
                    else:
                        nc.scalar.copy(dst, psa[t][:])

            a_wave(0, range(8))
            a_wave(1, range(0, 4))
            a_wave(1, range(4, 8))

            # ====== phase Q: q''T = A^T-contraction with xqT, + g ======
            def q_wave(nb, ms_):
                psq = {m: psp.tile([128, 512], F32, tag=f"ps{m}",
                                   name=f"psQ{nb}_{m}") for m in ms_}
                for t in range(EK):
                    for m in ms_:
                        nc.tensor.matmul(
                            psq[m][:],
                            a_tiles[t][:, m * 128:(m + 1) * 128],
                            xq_tiles[t][:, nb * 512:(nb + 1) * 512],
                            start=(t == 0), stop=(t == EK - 1))
                for m in ms_:
                    dst = qt_tiles[m][:, nb * 512:(nb + 1) * 512]
                    if m % 2 == 0:
                        nc.vector.tensor_scalar_add(
                            dst, psq[m][:], g_t[:, m:m + 1])
                    else:
                        nc.scalar.activation(
                            dst, psq[m][:], AF.Identity,
                            bias=g_t[:, m:m + 1])

            q_wave(0, range(8))
            q_wave(1, range(0, 4))
            q_wave(1, range(4, 8))

        # ====== phase D: attention, blocked over s_q ======
        with tc.tile_pool(name="expp", bufs=1) as expp, \
             tc.tile_pool(name="ztp", bufs=2) as ztp, \
             tc.tile_pool(name="otp", bufs=1) as otp, \
             tc.tile_pool(name="partp", bufs=2) as partp, \
             tc.tile_pool(name="rcp", bufs=2) as rcp:
            for blk in range(NBLK):
                q0 = blk * BQ
                # scoresT[s_k, blk] -> exp (bf16)
                exps = []
                for m in range(MK):
                    ps = psp.tile([128, BQ], F32, tag=f"ps{m % 2}",
                                  name=f"psS{blk}_{m}")
                    for k in range(EK):
                        nc.tensor.matmul(
                            ps[:],
                            xk_tiles[k][:, m * 128:(m + 1) * 128],
                            qt_tiles[k][:, q0:q0 + BQ],
                            start=(k == 0), stop=(k == EK - 1))
                    et = expp.tile([128, BQ], BF16, tag=f"exp{m}",
                                   name=f"exp{blk}_{m}")
                    nc.scalar.activation(et[:], ps[:], AF.Exp, scale=INV_SCALE)
                    exps.append(et)

                # partial sums over s_k tiles (DVE chain), overlaps ZT below
                part = partp.tile([128, BQ], F32, tag="part",
                                  name=f"part{blk}")
                nc.vector.tensor_add(part[:], exps[0][:], exps[1][:])
                for m in range(2, MK - 1):
                    nc.vector.tensor_add(part[:], part[:], exps[m][:])
                part_r = partp.tile([128, BQ], mybir.dt.float32r, tag="part_r",
                                    name=f"part_r{blk}")
                nc.vector.tensor_add(part_r[:], part[:], exps[MK - 1][:])

                # Z^T[e, i] = sum_j Xv[j, e] expT[j, i]
                zts = []
                for e_ in range(EK):
                    ps = psp.tile([128, BQ], F32, tag=f"ps{2 + e_ % 2}",
                                  name=f"psZ{blk}_{e_}")
                    for m in range(MK):
                        nc.tensor.matmul(
                            ps[:],
                            xv_tiles[m][:, e_ * 128:(e_ + 1) * 128],
                            exps[m][:],
                            start=(m == 0), stop=(m == MK - 1))
                    zt = ztp.tile([128, BQ], BF16, tag=f"zt{e_}",
                                  name=f"zt{blk}_{e_}")
                    nc.scalar.copy(zt[:], ps[:])
                    zts.append(zt)

                # partition-reduce of part_r via 256-wide ones-matmuls
                # (real-size matmuls keep the PE p-state stretch alive)
                recips = []
                for sh in range(2):
                    pssum = psp.tile([128, 512], F32, tag=f"ps{6 + sh}",
                                     name=f"psSum{blk}_{sh}")
                    for sl in range(2):
                        s = sh * 2 + sl
                        nc.tensor.matmul(
                            pssum[:, sl * 256:(sl + 1) * 256],
                            part_r[:, s * 128:(s + 1) * 128],
                            ones_r[:].bitcast(mybir.dt.float32r),
                            start=True, stop=True)
                    for sl in range(2):
                        s = sh * 2 + sl
                        rc = rcp.tile([128, 1], F32, tag=f"rc{s}",
                                      name=f"rc{blk}_{s}")
                        nc.vector.reciprocal(
                            rc[:], pssum[:, sl * 256:sl * 256 + 1])
                        recips.append(rc)

                # O = Z @ Wv, normalize by recip, + bv, DMA out.
                # The very last i-tile uses 256-wide PSUM groups so its
                # post-processing + DMAs overlap the final matmuls.
                for it in range(BQ // 128):
                    ot = otp.tile([128, E], F32, tag=f"ot{it}",
                                  name=f"ot{blk}_{it}")
                    last_it = (blk == NBLK - 1 and it == BQ // 128 - 1)
                    cw = 256 if last_it else 512
                    for ci, f0 in enumerate(range(0, E, cw)):
                        ps = psp.tile([128, cw], F32, tag=f"ps{4 + ci % 2}",
                                      name=f"psO{blk}_{it}_{ci}")
                        for e_ in range(EK):
                            nc.tensor.matmul(
                                ps[:],
                                zts[e_][:, it * 128:(it + 1) * 128],
                                wv_tiles[e_][:, f0:f0 + cw],
                                start=(e_ == 0), stop=(e_ == EK - 1))
                        nc.scalar.activation(
                            ot[:, f0:f0 + cw], ps[:],
                            AF.Copy, scale=recips[it][:])
                        nc.vector.tensor_add(
                            ot[:, f0:f0 + cw],
                            ot[:, f0:f0 + cw],
                            bv_bc[:, f0:f0 + cw])
                        nc.sync.dma_start(
                            out[q0 + it * 128:q0 + (it + 1) * 128,
                                f0:f0 + cw],
                            ot[:, f0:f0 + cw])

    nc.compile()
    return nc


def _get_nc():
    if "nc" not in _cached:
        _cached["nc"] = _build()
    return _cached["nc"]


def _bf16(a):
    return np.ascontiguousarray(np.asarray(a, dtype=np.float32)).astype(
        ml_dtypes.bfloat16)


def kernel(query, key, value, Wq, bq, Wk, bk, Wv, bv, **kw):
    query = np.asarray(query, dtype=np.float32)
    key = np.asarray(key, dtype=np.float32)
    value = np.asarray(value, dtype=np.float32)
    Wq = np.asarray(Wq, dtype=np.float32)
    Wk = np.asarray(Wk, dtype=np.float32)
    Wv = np.asarray(Wv, dtype=np.float32)
    bq = np.asarray(bq, dtype=np.float32)
    bv = np.asarray(bv, dtype=np.float32)

    wqT_h = _bf16(Wq.T)
    wkT_h = _bf16(Wk.T)
    wv_h = _bf16(Wv)
    g = Wk @ bq                       # [E]; bk cancels in softmax
    g_h = np.ascontiguousarray(g.reshape(EK, 128).T).astype(np.float32)
    bv_h = np.ascontiguousarray(bv.reshape(1, E))

    keyT = {b: _bf16(key[b].T) for b in range(B)}
    valN = {b: _bf16(value[b]) for b in range(B)}

    in_maps = []
    for c in range(N_CORES):
        b, h = divmod(c, 2)
        qT = _bf16(query[b, h * SQ:(h + 1) * SQ, :].T)
        in_maps.append({
            "wqT": wqT_h, "wkT": wkT_h, "xqT": qT,
            "xkT": keyT[b], "xv": valN[b], "wv": wv_h,
            "gh": g_h, "bvh": bv_h,
        })

    nc = _get_nc()
    res = bass_utils.run_bass_kernel_spmd(
        nc, in_maps, core_ids=list(range(N_CORES)), **kw)

    full = np.empty((B, S, E), dtype=np.float32)
    for c in range(N_CORES):
        b, h = divmod(c, 2)
        full[b, h * SQ:(h + 1) * SQ, :] = res.results[c]["out"]
    kernel.last_results = res
    return full


# revision 34
# speedup vs baseline: 1.0048x; 1.0022x over previous
"""Trainium2 Bass kernel for single-head attention model.

Reference computation (B=4, S=2048, E=1024, fp32):
    q = query @ Wq + bq;  k = key @ Wk + bk;  v = value @ Wv + bv
    scores = (q @ k^T) / sqrt(E)
    out = softmax(scores, axis=-1) @ v

Sharding: 8 cores; core c handles batch b = c // 2, query-row half
h = c % 2 (1024 q-rows). No collectives.

Algebraic restructure (saves ~23% of the MACs vs the direct form):
    scores_ij = x^q_i A x^k_j + g.x^k_j (+ row-const terms that cancel
    in softmax), where A = Wq Wk^T and g = Wk bq (host-computed).
    bk drops out entirely.  On the value side,
    out = softmax(scores) @ (Xv Wv + bv) = (attn @ Xv) @ Wv + bv
    since attn rows sum to 1 — Wv is applied to only the core's own
    1024 q rows instead of all 2048 kv rows.

Per-core matmul work (128x128 PE, 1 cycle/row at free>=256):
    A = WqWk^T (65536 cyc) ; q'T = A^T-contract with xqT (65536)
    scoresT (131072) ; Z^T = Xv^T-contract with exp (131072)
    O = Z Wv (65536)  => 458752 cycles ~= 191us @2.4GHz.

All matmul inputs are bf16 (host-converted); PSUM accumulates f32.
exp/Z intermediates stored bf16.  Softmax sums: DVE partial-sum chain
over the 16 key tiles, then a 256-wide ones-matmul per 128-row slice
for the partition reduction (wide enough not to break the PE p-state
stretch).  A 14-matmul PE warm-up bridges the initial DMA lead-in so
real matmuls start at full clock.  One shared 8-tag PSUM pool spans
all phases (no pool release/alloc barriers).
"""

import sys

sys.path.insert(0, "/opt/trn_rl_repo")

from contextlib import ExitStack

import ml_dtypes
import numpy as np

import concourse.mybir as mybir
import concourse.tile as tile
from concourse import bacc, bass_utils

BF16 = mybir.dt.bfloat16
F32 = mybir.dt.float32
AF = mybir.ActivationFunctionType

B, S, E = 4, 2048, 1024
N_CORES = 8
SQ = S // 2          # q rows per core
BQ = 512             # s_q block width in attention phase
NBLK = SQ // BQ      # 2 blocks
EK = E // 128        # 8 tiles over e/a/c dims
MK = S // 128        # 16 s_k tiles
INV_SCALE = 1.0 / float(np.sqrt(E))

_cached = {}


def _build():
    nc = bacc.Bacc("TRN2", target_bir_lowering=False, debug=False,
                   num_devices=N_CORES)

    # host pre-transposed / pre-converted inputs (all bf16 except consts)
    wqT = nc.dram_tensor("wqT", [E, E], BF16, kind="ExternalInput").ap()
    wkT = nc.dram_tensor("wkT", [E, E], BF16, kind="ExternalInput").ap()
    xqT = nc.dram_tensor("xqT", [E, SQ], BF16, kind="ExternalInput").ap()
    xkT = nc.dram_tensor("xkT", [E, S], BF16, kind="ExternalInput").ap()
    xv = nc.dram_tensor("xv", [S, E], BF16, kind="ExternalInput").ap()
    wv = nc.dram_tensor("wv", [E, E], BF16, kind="ExternalInput").ap()
    # g = Wk @ bq arranged g_h[p, t] = g[t*128 + p]
    gh = nc.dram_tensor("gh", [128, EK], F32, kind="ExternalInput").ap()
    bvh = nc.dram_tensor("bvh", [1, E], F32, kind="ExternalInput").ap()
    out = nc.dram_tensor("out", [SQ, E], F32, kind="ExternalOutput").ap()

    with tile.TileContext(nc) as tc, ExitStack() as top:
        # ---- long-lived pools ----
        consts = top.enter_context(tc.tile_pool(name="consts", bufs=1))
        qtpool = top.enter_context(tc.tile_pool(name="qtpool", bufs=1))
        xkpool = top.enter_context(tc.tile_pool(name="xkpool", bufs=1))
        xvpool = top.enter_context(tc.tile_pool(name="xvpool", bufs=1))
        wvpool = top.enter_context(tc.tile_pool(name="wvpool", bufs=1))

        # single shared PSUM pool: 8 tags x [128,512]f32 = 8 banks; shared
        # tags across phases avoid pool release/alloc barriers entirely
        psp = top.enter_context(tc.tile_pool(name="psp", bufs=1, space="PSUM"))

        qt_tiles = [qtpool.tile([128, SQ], BF16, tag=f"qt{m}", name=f"qt{m}")
                    for m in range(EK)]
        xk_tiles = [xkpool.tile([128, S], BF16, tag=f"xk{k}", name=f"xk{k}")
                    for k in range(EK)]
        xv_tiles = [xvpool.tile([128, E], BF16, tag=f"xv{m}", name=f"xv{m}")
                    for m in range(MK)]
        wv_tiles = [wvpool.tile([128, E], BF16, tag=f"wv{k}", name=f"wv{k}")
                    for k in range(EK)]

        with tc.tile_pool(name="wqwk", bufs=1) as wqwkp, \
             tc.tile_pool(name="apool", bufs=1) as apool, \
             tc.tile_pool(name="xqpool", bufs=1) as xqpool:
            wq_t = [wqwkp.tile([128, E], BF16, tag=f"wq{c}", name=f"wq{c}")
                    for c in range(EK)]
            wk_t = [wqwkp.tile([128, E], BF16, tag=f"wk{c}", name=f"wk{c}")
                    for c in range(EK)]
            a_tiles = [apool.tile([128, E], BF16, tag=f"a{t}", name=f"a{t}")
                       for t in range(EK)]
            xq_tiles = [xqpool.tile([128, SQ], BF16, tag=f"xq{t}", name=f"xq{t}")
                        for t in range(EK)]

            ones_r = consts.tile([128, 256], F32)
            ones_f32r = ones_r[:].bitcast(mybir.dt.float32r)
            ones_b = consts.tile([128, 128], BF16)

            # ---- PE warm-up: keep the tensor engine busy through the DMA
            # lead-in so the p-state ramp completes before real work.
            # Reads ones_r UNINITIALIZED on purpose (values never consumed);
            # the memset below is WAR-ordered after the warm-up reads and
            # completes long before the sums-matmuls need real ones. ----
            warm = psp.tile([128, 256], F32, tag="ps0", name="warm")
            for _ in range(14):
                nc.tensor.matmul(warm[:], ones_f32r[:, 0:128],
                                 ones_f32r, start=True, stop=True)
            nc.vector.memset(ones_r[:], 1.0)
            nc.vector.memset(ones_b[:], 1.0)

            # ---- DMA issue order = consumption order ----
            # wq full tiles + wk first halves feed phase A's nb=0 wave
            for c in range(EK):
                nc.sync.dma_start(wq_t[c][:], wqT[c * 128:(c + 1) * 128, :])
                nc.sync.dma_start(wk_t[c][:, 0:512],
                                  wkT[c * 128:(c + 1) * 128, 0:512])
            for c in range(EK):
                nc.sync.dma_start(wk_t[c][:, 512:1024],
                                  wkT[c * 128:(c + 1) * 128, 512:1024])
            g_t = consts.tile([128, EK], F32)
            nc.sync.dma_start(g_t[:], gh)
            bv_row = consts.tile([1, E], F32)
            nc.sync.dma_start(bv_row[:], bvh)
            bv_bc = consts.tile([128, E], F32)
            nc.gpsimd.partition_broadcast(bv_bc[:], bv_row[:])
            for t in range(EK):
                nc.sync.dma_start(xq_tiles[t][:], xqT[t * 128:(t + 1) * 128, :])
            for k in range(EK):
                nc.sync.dma_start(xk_tiles[k][:], xkT[k * 128:(k + 1) * 128, :])
            for m in range(MK):
                nc.sync.dma_start(xv_tiles[m][:], xv[m * 128:(m + 1) * 128, :])
            for k in range(EK):
                nc.sync.dma_start(wv_tiles[k][:], wv[k * 128:(k + 1) * 128, :])

            # ====== phase A: A = Wq Wk^T  (c-outer PSUM waves; the final
            # half-waves let next-phase matmuls overlap the copy tail) ======
            def a_wave(nb, ts_):
                psa = {t: psp.tile([128, 512], F32, tag=f"ps{t}",
                                   name=f"psA{nb}_{t}") for t in ts_}
                for c in range(EK):
                    for t in ts_:
                        nc.tensor.matmul(
                            psa[t][:],
                            wq_t[c][:, t * 128:(t + 1) * 128],
                            wk_t[c][:, nb * 512:(nb + 1) * 512],
                            start=(c == 0), stop=(c == EK - 1))
                # drain copies split across DVE/Act
                for t in ts_:
                    dst = a_tiles[t][:, nb * 512:(nb + 1) * 512]
                    if t % 2 == 0:
                        nc.vector.tensor_scalar_add(dst, psa[t][:], 0.0)
                    else:
                        nc.scalar.copy(dst, psa[t][:])

            a_wave(0, range(8))
            a_wave(1, range(0, 4))
            a_wave(1, range(4, 8))

            # ====== phase Q: q''T = A^T-contraction with xqT, + g ======
            def q_wave(nb, ms_):
                psq = {m: psp.tile([128, 512], F32, tag=f"ps{m}",
                                   name=f"psQ{nb}_{m}") for m in ms_}
                for t in range(EK):
                    for m in ms_:
                        nc.tensor.matmul(
                            psq[m][:],
                            a_tiles[t][:, m * 128:(m + 1) * 128],
                            xq_tiles[t][:, nb * 512:(nb + 1) * 512],
                            start=(t == 0), stop=(t == EK - 1))
                for m in ms_:
                    dst = qt_tiles[m][:, nb * 512:(nb + 1) * 512]
                    if m % 2 == 0:
                        nc.vector.tensor_scalar_add(
                            dst, psq[m][:], g_t[:, m:m + 1])
                    else:
                        nc.scalar.activation(
                            dst, psq[m][:], AF.Identity,
                            bias=g_t[:, m:m + 1])

            q_wave(0, range(8))
            q_wave(1, range(0, 4))
            q_wave(1, range(4, 8))

        # ====== phase D: attention, blocked over s_q ======
        with tc.tile_pool(name="expp", bufs=1) as expp, \
             tc.tile_pool(name="ztp", bufs=2) as ztp, \
             tc.tile_pool(name="otp", bufs=1) as otp, \
             tc.tile_pool(name="partp", bufs=2) as partp, \
             tc.tile_pool(name="rcp", bufs=2) as rcp:
            for blk in range(NBLK):
                q0 = blk * BQ
                # scoresT[s_k, blk] -> exp (bf16)
                exps = []
                for m in range(MK):
                    ps = psp.tile([128, BQ], F32, tag=f"ps{m % 2}",
                                  name=f"psS{blk}_{m}")
                    for k in range(EK):
                        nc.tensor.matmul(
                            ps[:],
                            xk_tiles[k][:, m * 128:(m + 1) * 128],
                            qt_tiles[k][:, q0:q0 + BQ],
                            start=(k == 0), stop=(k == EK - 1))
                    et = expp.tile([128, BQ], BF16, tag=f"exp{m}",
                                   name=f"exp{blk}_{m}")
                    nc.scalar.activation(et[:], ps[:], AF.Exp, scale=INV_SCALE)
                    exps.append(et)

                # partial sums over s_k tiles (DVE chain), overlaps ZT below
                part = partp.tile([128, BQ], F32, tag="part",
                                  name=f"part{blk}")
                nc.vector.tensor_add(part[:], exps[0][:], exps[1][:])
                for m in range(2, MK - 1):
                    nc.vector.tensor_add(part[:], part[:], exps[m][:])
                part_r = partp.tile([128, BQ], BF16, tag="part_r",
                                    name=f"part_r{blk}")
                nc.vector.tensor_add(part_r[:], part[:], exps[MK - 1][:])

                # Z^T[e, i] = sum_j Xv[j, e] expT[j, i]
                zts = []
                for e_ in range(EK):
                    ps = psp.tile([128, BQ], F32, tag=f"ps{2 + e_ % 2}",
                                  name=f"psZ{blk}_{e_}")
                    for m in range(MK):
                        nc.tensor.matmul(
                            ps[:],
                            xv_tiles[m][:, e_ * 128:(e_ + 1) * 128],
                            exps[m][:],
                            start=(m == 0), stop=(m == MK - 1))
                    zt = ztp.tile([128, BQ], BF16, tag=f"zt{e_}",
                                  name=f"zt{blk}_{e_}")
                    nc.scalar.copy(zt[:], ps[:])
                    zts.append(zt)

                # partition-reduce of part_r via 128-wide bf16 ones-matmuls
                # (bf16 runs 1 cyc/row at any width; 128 is still wide
                # enough not to break the PE p-state stretch)
                pssum = psp.tile([128, 512], F32, tag="ps6",
                                 name=f"psSum{blk}")
                recips = []
                for s in range(BQ // 128):
                    nc.tensor.matmul(
                        pssum[:, s * 128:(s + 1) * 128],
                        part_r[:, s * 128:(s + 1) * 128],
                        ones_b[:], start=True, stop=True)
                for s in range(BQ // 128):
                    rc = rcp.tile([128, 1], F32, tag=f"rc{s}",
                                  name=f"rc{blk}_{s}")
                    nc.vector.reciprocal(
                        rc[:], pssum[:, s * 128:s * 128 + 1])
                    recips.append(rc)

                # O = Z @ Wv, normalize by recip, + bv, DMA out.
                # The very last i-tile uses 256-wide PSUM groups so its
                # post-processing + DMAs overlap the final matmuls.
                for it in range(BQ // 128):
                    ot = otp.tile([128, E], F32, tag=f"ot{it}",
                                  name=f"ot{blk}_{it}")
                    last_it = (blk == NBLK - 1 and it == BQ // 128 - 1)
                    cw = 256 if last_it else 512
                    for ci, f0 in enumerate(range(0, E, cw)):
                        ps = psp.tile([128, cw], F32, tag=f"ps{4 + ci % 2}",
                                      name=f"psO{blk}_{it}_{ci}")
                        for e_ in range(EK):
                            nc.tensor.matmul(
                                ps[:],
                                zts[e_][:, it * 128:(it + 1) * 128],
                                wv_tiles[e_][:, f0:f0 + cw],
                                start=(e_ == 0), stop=(e_ == EK - 1))
                        nc.scalar.activation(
                            ot[:, f0:f0 + cw], ps[:],
                            AF.Copy, scale=recips[it][:])
                        nc.vector.tensor_add(
                            ot[:, f0:f0 + cw],
                            ot[:, f0:f0 + cw],
                            bv_bc[:, f0:f0 + cw])
                        nc.sync.dma_start(
                            out[q0 + it * 128:q0 + (it + 1) * 128,
                                f0:f0 + cw],
                            ot[:, f0:f0 + cw])

    nc.compile()
    return nc


def _get_nc():
    if "nc" not in _cached:
        _cached["nc"] = _build()
    return _cached["nc"]


def _bf16(a):
    return np.ascontiguousarray(np.asarray(a, dtype=np.float32)).astype(
        ml_dtypes.bfloat16)


def kernel(query, key, value, Wq, bq, Wk, bk, Wv, bv, **kw):
    query = np.asarray(query, dtype=np.float32)
    key = np.asarray(key, dtype=np.float32)
    value = np.asarray(value, dtype=np.float32)
    Wq = np.asarray(Wq, dtype=np.float32)
    Wk = np.asarray(Wk, dtype=np.float32)
    Wv = np.asarray(Wv, dtype=np.float32)
    bq = np.asarray(bq, dtype=np.float32)
    bv = np.asarray(bv, dtype=np.float32)

    wqT_h = _bf16(Wq.T)
    wkT_h = _bf16(Wk.T)
    wv_h = _bf16(Wv)
    g = Wk @ bq                       # [E]; bk cancels in softmax
    g_h = np.ascontiguousarray(g.reshape(EK, 128).T).astype(np.float32)
    bv_h = np.ascontiguousarray(bv.reshape(1, E))

    keyT = {b: _bf16(key[b].T) for b in range(B)}
    valN = {b: _bf16(value[b]) for b in range(B)}

    in_maps = []
    for c in range(N_CORES):
        b, h = divmod(c, 2)
        qT = _bf16(query[b, h * SQ:(h + 1) * SQ, :].T)
        in_maps.append({
            "wqT": wqT_h, "wkT": wkT_h, "xqT": qT,
            "xkT": keyT[b], "xv": valN[b], "wv": wv_h,
            "gh": g_h, "bvh": bv_h,
        })

    nc = _get_nc()
    res = bass_utils.run_bass_kernel_spmd(
        nc, in_maps, core_ids=list(range(N_CORES)), **kw)

    full = np.empty((B, S, E), dtype=np.float32)
    for c in range(N_CORES):
        b, h = divmod(c, 2)
        full[b, h * SQ:(h + 1) * SQ, :] = res.results[c]["out"]
    kernel.last_results = res
    return full


# revision 35
# speedup vs baseline: 1.0064x; 1.0016x over previous
"""Trainium2 Bass kernel for single-head attention model.

Reference computation (B=4, S=2048, E=1024, fp32):
    q = query @ Wq + bq;  k = key @ Wk + bk;  v = value @ Wv + bv
    scores = (q @ k^T) / sqrt(E)
    out = softmax(scores, axis=-1) @ v

Sharding: 8 cores; core c handles batch b = c // 2, query-row half
h = c % 2 (1024 q-rows). No collectives.

Algebraic restructure (saves ~23% of the MACs vs the direct form):
    scores_ij = x^q_i A x^k_j + g.x^k_j (+ row-const terms that cancel
    in softmax), where A = Wq Wk^T and g = Wk bq (host-computed).
    bk drops out entirely.  On the value side,
    out = softmax(scores) @ (Xv Wv + bv) = (attn @ Xv) @ Wv + bv
    since attn rows sum to 1 — Wv is applied to only the core's own
    1024 q rows instead of all 2048 kv rows.

Per-core matmul work (128x128 PE, 1 cycle/row at free>=256):
    A = WqWk^T (65536 cyc) ; q'T = A^T-contract with xqT (65536)
    scoresT (131072) ; Z^T = Xv^T-contract with exp (131072)
    O = Z Wv (65536)  => 458752 cycles ~= 191us @2.4GHz.

All matmul inputs are bf16 (host-converted); PSUM accumulates f32.
exp/Z intermediates stored bf16.  Softmax sums: DVE partial-sum chain
over the 16 key tiles, then a 256-wide ones-matmul per 128-row slice
for the partition reduction (wide enough not to break the PE p-state
stretch).  A 14-matmul PE warm-up bridges the initial DMA lead-in so
real matmuls start at full clock.  One shared 8-tag PSUM pool spans
all phases (no pool release/alloc barriers).
"""

import sys

sys.path.insert(0, "/opt/trn_rl_repo")

from contextlib import ExitStack

import ml_dtypes
import numpy as np

import concourse.mybir as mybir
import concourse.tile as tile
from concourse import bacc, bass_utils

BF16 = mybir.dt.bfloat16
F32 = mybir.dt.float32
AF = mybir.ActivationFunctionType

B, S, E = 4, 2048, 1024
N_CORES = 8
SQ = S // 2          # q rows per core
BQ = 512             # s_q block width in attention phase
NBLK = SQ // BQ      # 2 blocks
EK = E // 128        # 8 tiles over e/a/c dims
MK = S // 128        # 16 s_k tiles
INV_SCALE = 1.0 / float(np.sqrt(E))

_cached = {}


def _build():
    nc = bacc.Bacc("TRN2", target_bir_lowering=False, debug=False,
                   num_devices=N_CORES)

    # host pre-transposed / pre-converted inputs (all bf16 except consts)
    wqT = nc.dram_tensor("wqT", [E, E], BF16, kind="ExternalInput").ap()
    wkT = nc.dram_tensor("wkT", [E, E], BF16, kind="ExternalInput").ap()
    xqT = nc.dram_tensor("xqT", [E, SQ], BF16, kind="ExternalInput").ap()
    xkT = nc.dram_tensor("xkT", [E, S], BF16, kind="ExternalInput").ap()
    xv = nc.dram_tensor("xv", [S, E], BF16, kind="ExternalInput").ap()
    wv = nc.dram_tensor("wv", [E, E], BF16, kind="ExternalInput").ap()
    # g = Wk @ bq arranged g_h[p, t] = g[t*128 + p]
    gh = nc.dram_tensor("gh", [128, EK], F32, kind="ExternalInput").ap()
    bvh = nc.dram_tensor("bvh", [1, E], F32, kind="ExternalInput").ap()
    out = nc.dram_tensor("out", [SQ, E], F32, kind="ExternalOutput").ap()

    with tile.TileContext(nc) as tc, ExitStack() as top:
        # ---- long-lived pools ----
        consts = top.enter_context(tc.tile_pool(name="consts", bufs=1))
        qtpool = top.enter_context(tc.tile_pool(name="qtpool", bufs=1))
        xkpool = top.enter_context(tc.tile_pool(name="xkpool", bufs=1))
        xvpool = top.enter_context(tc.tile_pool(name="xvpool", bufs=1))
        wvpool = top.enter_context(tc.tile_pool(name="wvpool", bufs=1))

        # single shared PSUM pool: 8 tags x [128,512]f32 = 8 banks; shared
        # tags across phases avoid pool release/alloc barriers entirely
        psp = top.enter_context(tc.tile_pool(name="psp", bufs=1, space="PSUM"))

        qt_tiles = [qtpool.tile([128, SQ], BF16, tag=f"qt{m}", name=f"qt{m}")
                    for m in range(EK)]
        xk_tiles = [xkpool.tile([128, S], BF16, tag=f"xk{k}", name=f"xk{k}")
                    for k in range(EK)]
        xv_tiles = [xvpool.tile([128, E], BF16, tag=f"xv{m}", name=f"xv{m}")
                    for m in range(MK)]
        wv_tiles = [wvpool.tile([128, E], BF16, tag=f"wv{k}", name=f"wv{k}")
                    for k in range(EK)]

        with tc.tile_pool(name="wqwk", bufs=1) as wqwkp, \
             tc.tile_pool(name="apool", bufs=1) as apool, \
             tc.tile_pool(name="xqpool", bufs=1) as xqpool:
            wq_t = [wqwkp.tile([128, E], BF16, tag=f"wq{c}", name=f"wq{c}")
                    for c in range(EK)]
            wk_t = [wqwkp.tile([128, E], BF16, tag=f"wk{c}", name=f"wk{c}")
                    for c in range(EK)]
            a_tiles = [apool.tile([128, E], BF16, tag=f"a{t}", name=f"a{t}")
                       for t in range(EK)]
            xq_tiles = [xqpool.tile([128, SQ], BF16, tag=f"xq{t}", name=f"xq{t}")
                        for t in range(EK)]

            ones_r = consts.tile([128, 256], F32)
            ones_f32r = ones_r[:].bitcast(mybir.dt.float32r)
            ones_b = consts.tile([128, 32], BF16)

            # ---- PE warm-up: keep the tensor engine busy through the DMA
            # lead-in so the p-state ramp completes before real work.
            # Reads ones_r UNINITIALIZED on purpose (values never consumed);
            # the memset below is WAR-ordered after the warm-up reads and
            # completes long before the sums-matmuls need real ones. ----
            warm = psp.tile([128, 256], F32, tag="ps0", name="warm")
            for _ in range(14):
                nc.tensor.matmul(warm[:], ones_f32r[:, 0:128],
                                 ones_f32r, start=True, stop=True)
            nc.vector.memset(ones_r[:], 1.0)
            nc.vector.memset(ones_b[:], 1.0)

            # ---- DMA issue order = consumption order ----
            # wq full tiles + wk first halves feed phase A's nb=0 wave
            for c in range(EK):
                nc.sync.dma_start(wq_t[c][:], wqT[c * 128:(c + 1) * 128, :])
                nc.sync.dma_start(wk_t[c][:, 0:512],
                                  wkT[c * 128:(c + 1) * 128, 0:512])
            for c in range(EK):
                nc.sync.dma_start(wk_t[c][:, 512:1024],
                                  wkT[c * 128:(c + 1) * 128, 512:1024])
            g_t = consts.tile([128, EK], F32)
            nc.sync.dma_start(g_t[:], gh)
            bv_row = consts.tile([1, E], F32)
            nc.sync.dma_start(bv_row[:], bvh)
            bv_bc = consts.tile([128, E], F32)
            nc.gpsimd.partition_broadcast(bv_bc[:], bv_row[:])
            for t in range(EK):
                nc.sync.dma_start(xq_tiles[t][:], xqT[t * 128:(t + 1) * 128, :])
            for k in range(EK):
                nc.sync.dma_start(xk_tiles[k][:], xkT[k * 128:(k + 1) * 128, :])
            for m in range(MK):
                nc.sync.dma_start(xv_tiles[m][:], xv[m * 128:(m + 1) * 128, :])
            for k in range(EK):
                nc.sync.dma_start(wv_tiles[k][:], wv[k * 128:(k + 1) * 128, :])

            # ====== phase A: A = Wq Wk^T  (c-outer PSUM waves; the final
            # half-waves let next-phase matmuls overlap the copy tail) ======
            def a_wave(nb, ts_):
                psa = {t: psp.tile([128, 512], F32, tag=f"ps{t}",
                                   name=f"psA{nb}_{t}") for t in ts_}
                for c in range(EK):
                    for t in ts_:
                        nc.tensor.matmul(
                            psa[t][:],
                            wq_t[c][:, t * 128:(t + 1) * 128],
                            wk_t[c][:, nb * 512:(nb + 1) * 512],
                            start=(c == 0), stop=(c == EK - 1))
                # drain copies split across DVE/Act
                for t in ts_:
                    dst = a_tiles[t][:, nb * 512:(nb + 1) * 512]
                    if t % 2 == 0:
                        nc.vector.tensor_scalar_add(dst, psa[t][:], 0.0)
                    else:
                        nc.scalar.copy(dst, psa[t][:])

            a_wave(0, range(8))
            a_wave(1, range(0, 4))
            a_wave(1, range(4, 8))

            # ====== phase Q: q''T = A^T-contraction with xqT, + g ======
            def q_wave(nb, ms_):
                psq = {m: psp.tile([128, 512], F32, tag=f"ps{m}",
                                   name=f"psQ{nb}_{m}") for m in ms_}
                for t in range(EK):
                    for m in ms_:
                        nc.tensor.matmul(
                            psq[m][:],
                            a_tiles[t][:, m * 128:(m + 1) * 128],
                            xq_tiles[t][:, nb * 512:(nb + 1) * 512],
                            start=(t == 0), stop=(t == EK - 1))
                for m in ms_:
                    dst = qt_tiles[m][:, nb * 512:(nb + 1) * 512]
                    if m % 2 == 0:
                        nc.vector.tensor_scalar_add(
                            dst, psq[m][:], g_t[:, m:m + 1])
                    else:
                        nc.scalar.activation(
                            dst, psq[m][:], AF.Identity,
                            bias=g_t[:, m:m + 1])

            q_wave(0, range(8))
            q_wave(1, range(0, 4))
            q_wave(1, range(4, 8))

        # ====== phase D: attention, blocked over s_q ======
        with tc.tile_pool(name="expp", bufs=1) as expp, \
             tc.tile_pool(name="ztp", bufs=2) as ztp, \
             tc.tile_pool(name="otp", bufs=1) as otp, \
             tc.tile_pool(name="partp", bufs=2) as partp, \
             tc.tile_pool(name="rcp", bufs=2) as rcp:
            for blk in range(NBLK):
                q0 = blk * BQ
                # scoresT[s_k, blk] -> exp (bf16)
                exps = []
                for m in range(MK):
                    ps = psp.tile([128, BQ], F32, tag=f"ps{m % 2}",
                                  name=f"psS{blk}_{m}")
                    for k in range(EK):
                        nc.tensor.matmul(
                            ps[:],
                            xk_tiles[k][:, m * 128:(m + 1) * 128],
                            qt_tiles[k][:, q0:q0 + BQ],
                            start=(k == 0), stop=(k == EK - 1))
                    et = expp.tile([128, BQ], BF16, tag=f"exp{m}",
                                   name=f"exp{blk}_{m}")
                    nc.scalar.activation(et[:], ps[:], AF.Exp, scale=INV_SCALE)
                    exps.append(et)

                # partial sums over s_k tiles (DVE chain), overlaps ZT below
                part = partp.tile([128, BQ], F32, tag="part",
                                  name=f"part{blk}")
                nc.vector.tensor_add(part[:], exps[0][:], exps[1][:])
                for m in range(2, MK - 1):
                    nc.vector.tensor_add(part[:], part[:], exps[m][:])
                part_r = partp.tile([128, BQ], BF16, tag="part_r",
                                    name=f"part_r{blk}")
                nc.vector.tensor_add(part_r[:], part[:], exps[MK - 1][:])

                # Z^T[e, i] = sum_j Xv[j, e] expT[j, i]
                zts = []
                for e_ in range(EK):
                    ps = psp.tile([128, BQ], F32, tag=f"ps{2 + e_ % 2}",
                                  name=f"psZ{blk}_{e_}")
                    for m in range(MK):
                        nc.tensor.matmul(
                            ps[:],
                            xv_tiles[m][:, e_ * 128:(e_ + 1) * 128],
                            exps[m][:],
                            start=(m == 0), stop=(m == MK - 1))
                    zt = ztp.tile([128, BQ], BF16, tag=f"zt{e_}",
                                  name=f"zt{blk}_{e_}")
                    nc.scalar.copy(zt[:], ps[:])
                    zts.append(zt)

                # partition-reduce of part_r via 128-wide bf16 ones-matmuls
                # (bf16 runs 1 cyc/row at any width; 128 is still wide
                # enough not to break the PE p-state stretch)
                pssum = psp.tile([128, 128], F32, tag="ps6",
                                 name=f"psSum{blk}")
                recips = []
                for s in range(BQ // 128):
                    nc.tensor.matmul(
                        pssum[:, s * 32:(s + 1) * 32],
                        part_r[:, s * 128:(s + 1) * 128],
                        ones_b[:], start=True, stop=True)
                for s in range(BQ // 128):
                    rc = rcp.tile([128, 1], F32, tag=f"rc{s}",
                                  name=f"rc{blk}_{s}")
                    nc.vector.reciprocal(
                        rc[:], pssum[:, s * 32:s * 32 + 1])
                    recips.append(rc)

                # O = Z @ Wv, normalize by recip, + bv, DMA out.
                # The very last i-tile uses 256-wide PSUM groups so its
                # post-processing + DMAs overlap the final matmuls.
                for it in range(BQ // 128):
                    ot = otp.tile([128, E], F32, tag=f"ot{it}",
                                  name=f"ot{blk}_{it}")
                    last_it = (blk == NBLK - 1 and it == BQ // 128 - 1)
                    cw = 256 if last_it else 512
                    for ci, f0 in enumerate(range(0, E, cw)):
                        ps = psp.tile([128, cw], F32, tag=f"ps{4 + ci % 2}",
                                      name=f"psO{blk}_{it}_{ci}")
                        for e_ in range(EK):
                            nc.tensor.matmul(
                                ps[:],
                                zts[e_][:, it * 128:(it + 1) * 128],
                                wv_tiles[e_][:, f0:f0 + cw],
                                start=(e_ == 0), stop=(e_ == EK - 1))
                        nc.scalar.activation(
                            ot[:, f0:f0 + cw], ps[:],
                            AF.Copy, scale=recips[it][:])
                        nc.vector.tensor_add(
                            ot[:, f0:f0 + cw],
                            ot[:, f0:f0 + cw],
                            bv_bc[:, f0:f0 + cw])
                        nc.sync.dma_start(
                            out[q0 + it * 128:q0 + (it + 1) * 128,
                                f0:f0 + cw],
                            ot[:, f0:f0 + cw])

    nc.compile()
    return nc


def _get_nc():
    if "nc" not in _cached:
        _cached["nc"] = _build()
    return _cached["nc"]


def _bf16(a):
    return np.ascontiguousarray(np.asarray(a, dtype=np.float32)).astype(
        ml_dtypes.bfloat16)


def kernel(query, key, value, Wq, bq, Wk, bk, Wv, bv, **kw):
    query = np.asarray(query, dtype=np.float32)
    key = np.asarray(key, dtype=np.float32)
    value = np.asarray(value, dtype=np.float32)
    Wq = np.asarray(Wq, dtype=np.float32)
    Wk = np.asarray(Wk, dtype=np.float32)
    Wv = np.asarray(Wv, dtype=np.float32)
    bq = np.asarray(bq, dtype=np.float32)
    bv = np.asarray(bv, dtype=np.float32)

    wqT_h = _bf16(Wq.T)
    wkT_h = _bf16(Wk.T)
    wv_h = _bf16(Wv)
    g = Wk @ bq                       # [E]; bk cancels in softmax
    g_h = np.ascontiguousarray(g.reshape(EK, 128).T).astype(np.float32)
    bv_h = np.ascontiguousarray(bv.reshape(1, E))

    keyT = {b: _bf16(key[b].T) for b in range(B)}
    valN = {b: _bf16(value[b]) for b in range(B)}

    in_maps = []
    for c in range(N_CORES):
        b, h = divmod(c, 2)
        qT = _bf16(query[b, h * SQ:(h + 1) * SQ, :].T)
        in_maps.append({
            "wqT": wqT_h, "wkT": wkT_h, "xqT": qT,
            "xkT": keyT[b], "xv": valN[b], "wv": wv_h,
            "gh": g_h, "bvh": bv_h,
        })

    nc = _get_nc()
    res = bass_utils.run_bass_kernel_spmd(
        nc, in_maps, core_ids=list(range(N_CORES)), **kw)

    full = np.empty((B, S, E), dtype=np.float32)
    for c in range(N_CORES):
        b, h = divmod(c, 2)
        full[b, h * SQ:(h + 1) * SQ, :] = res.results[c]["out"]
    kernel.last_results = res
    return full
